# revision 4
# baseline (speedup 1.0000x reference)
"""Multi-head causal attention (B=2, T=2048, C=1024, H=16, D=64) on 8 TRN2 cores.

Sharding: core c = (batch b = c//4, head-group hg = c%4): 4 heads of one batch
per core. Host concatenates heads / batches and normalizes (divide by softmax
sums) + transposes on the way out.

Per-core dataflow (all matmuls accumulate f32 in PSUM):
  1. q/k projections fp8 x fp8 DoubleRow: W chunk stationary [128c, 2, 128],
     x8^T moving [128c, 2, 512] -> Q~/K~ = x.Wq (un-downscaled) drained as
     fp8e4 into the DR scores layout: QT8/KT8[128p, 2, T] where partition
     p = 32h + (d&31) and the middle index is d>>5 (host permutes Wq/Wk
     columns so the projection writes this layout directly).
  2. v projection TRANSPOSED: x^T chunk stationary [128c, 128t] bf16, Wv
     moving [128c, 256hd] -> V[t, hd] directly (no PE transposes).
  3. Scores S^T[s, 4h, t]: fp8 DoubleRow, ALL FOUR heads concurrent via
     row-tiling (each head owns PE rows 32h..32h+31 with its d=64
     contraction 2-packed). One wave of 4 MMs per s-block. The 1/sqrt(C)
     softmax scale is folded into the exp (free affine / Schraudolph mult).
     Columns below the causal diagonal never computed (off trim).
  4. exp: per (s-block, head-pair) block. Short softmax rows (t<128) live in
     s-block 0 which is exact on ScalarE; other blocks split Scalar/DVE
     (DVE = 1-op Schraudolph: E_bf16bits = int16(S*(128/ln2)/32 + b)),
     balanced so both engines carry ~equal evacuation load. Diagonal 128x128
     gets its causal triangle zeroed by GpSimd affine_select (GpSimd is
     otherwise idle; keeps DVE free).
  5. AV: V[s,d] stationary per head, E^T moving; two heads of a pair run
     concurrently via col-tiling -> av^T[d-pair, t], already y^T layout.
     Softmax sums via 4 concurrent M=1 col-tiled matmuls into partitions
     {0,32,64,96}.
  6. Drains: y = av PSUM -> SBUF copy -> DMA. sums: ONE full-partition copy
     PSUM->SBUF then 4 single-partition DMAs (engine APs can't compact
     strided partitions, DMAs can address them).
  7. Host: y = (yt / sums) per head, transpose, concat.

Schedule: fused streaming pipeline per t-tile; next tile's projection closures
drain into the current tile's attention periods; AV lags scores by 2 s-blocks.
PSUM: scores 2x2 banks + av 2x1 + sums 1 + proj 1 = exactly 8 banks.
"""

import numpy as np

import concourse.mybir as mybir
import concourse.tile as tile
from concourse import bacc

B, T, C, H, D = 2, 2048, 1024, 16, 64
HPC = 4          # heads per core
NPAIR = 2        # head pairs per core
NCORES = 8
TT = 512         # t-tile
SB = 128         # s-block
NCH = C // 128   # contraction chunks
NTB = T // 128   # t-blocks (v-proj granularity)
F32 = mybir.dt.float32
BF16 = mybir.dt.bfloat16
F8 = mybir.dt.float8e4
I16 = mybir.dt.int16
WS = 32.0        # fp8 weight pre-scale (W*32 fits e4m3; drains divide it out)
SCALE = float(C) ** -0.5   # softmax scale, folded into exp

# exp(x) ~= bitcast_bf16(int16(x * 128/ln2 + (127*128 - c))): Schraudolph in
# bf16 bits. Scaled scores are in [-1, 1] so the int is ~16065..16435: no
# overflow, no sign issues. c calibrated offline.
SCH_K = float(2.0 ** 7 / np.log(2.0))
SCH_B = float(127 * 128 - 5.0)


# exp blocks (sb>0) are split along t across BOTH engines so each S bank
# recycles in ~half the single-engine block time -- the scores(sb+1) matmul
# waits on exp(sb) via the PSUM WAR, so exp block latency is the attention
# pipeline's critical path. Scalar gets the smaller share (its per-inst
# overhead is ~352cyc vs DVE ~100, and it also carries the y/sm drains).
EXP_SCALAR_FRAC = 0.34


def build_nc():
    nj = T // TT
    nc = bacc.Bacc("TRN2", target_bir_lowering=False, debug=False)
    xt = nc.dram_tensor("xt", [C, T], BF16, kind="ExternalInput")
    xt8 = nc.dram_tensor("xt8", [C, T], F8, kind="ExternalInput")
    wq = nc.dram_tensor("wq", [C, HPC * D], F8, kind="ExternalInput")
    wk = nc.dram_tensor("wk", [C, HPC * D], F8, kind="ExternalInput")
    wv = nc.dram_tensor("wv", [C, HPC * D], BF16, kind="ExternalInput")
    yt = nc.dram_tensor("yt", [HPC * D, T], F32, kind="ExternalOutput")
    sm = nc.dram_tensor("sm", [nj, HPC, TT], F32, kind="ExternalOutput")

    with tile.TileContext(nc) as tc:
        with (
            tc.tile_pool(name="wpool", bufs=1) as wpool,
            tc.tile_pool(name="xtp", bufs=2) as xtp,
            tc.tile_pool(name="epool", bufs=8) as epool,
            tc.tile_pool(name="ysb", bufs=4) as ysbp,
            tc.tile_pool(name="mixps", bufs=1, space="PSUM") as mixps,
            tc.tile_pool(name="spsum", bufs=2, space="PSUM") as spsum,
            tc.tile_pool(name="avps", bufs=2, space="PSUM") as avps,
            tc.tile_pool(name="smps", bufs=1, space="PSUM") as smps,
        ):
            ones1 = wpool.tile([128, 1], BF16)
            nc.vector.memset(ones1, 1.0)

            wq_sb = wpool.tile([128, NCH, HPC * D], F8, tag="wq")
            wk_sb = wpool.tile([128, NCH, HPC * D], F8, tag="wk")
            wv_sb = wpool.tile([128, NCH, HPC * D], BF16, tag="wv")
            w_sb = {"q": wq_sb, "k": wk_sb}

            # DR scores layout: partition 32h + (d&31), middle dim d>>5
            QT8 = wpool.tile([128, 2, T], F8, tag="qt8")
            KT8 = wpool.tile([128, 2, T], F8, tag="kt8")
            qk_dst = {"q": QT8, "k": KT8}
            V = wpool.tile([128, NTB, HPC * D], BF16, tag="v")

            def proj_closures(j):
                state = {}

                def do_load8():
                    # fp8 copy first (q/k proj gates the pipeline fill);
                    # chunk-pair granularity so the first matmuls only wait
                    # for the first 2 contraction chunks, not the full tile.
                    x8r = xt8.rearrange("(k p) t -> p k t", p=128)
                    xt8_sb = xtp.tile([128, NCH, TT], F8, tag="xts8",
                                      name=f"xts8_{j}")
                    for kk in range(0, NCH, 2):
                        nc.sync.dma_start(
                            out=xt8_sb[:, kk:kk + 2, :],
                            in_=x8r[:, kk:kk + 2, j * TT:(j + 1) * TT])
                    state["xt8"] = xt8_sb

                def do_load16():
                    xr = xt.rearrange("(k p) t -> p k t", p=128)
                    xt_sb = xtp.tile([128, NCH, TT], BF16, tag="xts",
                                     name=f"xts{j}")
                    for kk in range(0, NCH, 4):
                        nc.sync.dma_start(
                            out=xt_sb[:, kk:kk + 4, :],
                            in_=xr[:, kk:kk + 4, j * TT:(j + 1) * TT])
                    state["xt"] = xt_sb

                def do_qk(name, half):
                    # fp8e4 DoubleRow: two contraction chunks per matmul
                    pp = mixps.tile([128, TT], F32, tag="mix",
                                    name=f"pp_{name}{half}_{j}")
                    for kk in range(0, NCH, 2):
                        nc.tensor.matmul(
                            pp,
                            lhsT=w_sb[name][:, kk:kk + 2,
                                            half * 128:(half + 1) * 128],
                            rhs=state["xt8"][:, kk:kk + 2, :],
                            start=(kk == 0), stop=(kk == NCH - 2),
                            skip_group_check=True,
                            perf_mode=mybir.MatmulPerfMode.DoubleRow,
                        )
                    nc.vector.tensor_scalar(
                        qk_dst[name][:, half, j * TT:(j + 1) * TT], pp,
                        1.0 / WS, None, mybir.AluOpType.mult)

                def do_v(tb2):
                    vp = mixps.tile([128, TT], F32, tag="mix",
                                    name=f"vp{tb2}_{j}")
                    for half in range(2):
                        tl = tb2 * 2 + half
                        for kk in range(NCH):
                            nc.tensor.matmul(
                                vp[:, half * 256:(half + 1) * 256],
                                lhsT=state["xt"][:, kk, tl * 128:(tl + 1) * 128],
                                rhs=wv_sb[:, kk, :],
                                start=(kk == 0), stop=(kk == NCH - 1),
                                skip_group_check=True,
                            )
                    gtb = j * 4 + tb2 * 2
                    nc.vector.tensor_copy(
                        V[:, gtb:gtb + 2, :],
                        vp.rearrange("p (a b) -> p a b", a=2))

                ops = [lambda: do_qk("q", 0), lambda: do_qk("q", 1),
                       lambda: do_qk("k", 0), lambda: do_qk("k", 1),
                       lambda: do_v(0), lambda: do_v(1)]
                return do_load8, do_load16, ops

            def emit_attention(j, pending):
                n_sb = 4 * (j + 1)
                av_ps = {g: avps.tile([128, TT], F32, tag="avps",
                                      name=f"av{j}_{g}")
                         for g in range(NPAIR)}
                sm_ps = smps.tile([128, TT], F32, tag="smps", name=f"smps{j}")
                eg = {}
                LAG = 2

                def emit_scores(sb):
                    off = max(0, (sb - 4 * j) * SB)
                    S = {}
                    for g in range(NPAIR):
                        S[g] = spsum.tile([128, 2, TT], F32, tag="spsum",
                                          name=f"s{j}_{sb}_{g}")
                    # one wave: all 4 heads concurrent (row-tiled DR)
                    for h in range(HPC):
                        g, hh = h // 2, h % 2
                        hp = slice(32 * h, 32 * h + 32)
                        nc.tensor.matmul(
                            S[g][:, hh, off:],
                            lhsT=KT8[hp, :, sb * SB:(sb + 1) * SB],
                            rhs=QT8[hp, :, j * TT + off:(j + 1) * TT],
                            start=True, stop=True,
                            perf_mode=mybir.MatmulPerfMode.DoubleRow,
                            tile_position=(32 * h, 0),
                        )
                    for g in range(NPAIR):
                        e = epool.tile([128, 2, TT], BF16, tag="e",
                                       name=f"e{j}_{sb}_{g}")
                        if sb == 0:
                            nc.scalar.activation(
                                out=e[:, :, off:], in_=S[g][:, :, off:],
                                func=mybir.ActivationFunctionType.Exp,
                                scale=SCALE)
                        else:
                            w = TT - off
                            cs = off + max(8, int(w * EXP_SCALAR_FRAC) & ~3)
                            nc.scalar.activation(
                                out=e[:, :, off:cs], in_=S[g][:, :, off:cs],
                                func=mybir.ActivationFunctionType.Exp,
                                scale=SCALE)
                            nc.vector.tensor_scalar(
                                e.bitcast(I16)[:, :, cs:],
                                S[g][:, :, cs:],
                                SCH_K * SCALE, SCH_B,
                                mybir.AluOpType.mult, mybir.AluOpType.add)
                        if sb >= 4 * j:  # diagonal block: causal triangle
                            nc.gpsimd.affine_select(
                                out=e[:, :, off:off + SB],
                                in_=e[:, :, off:off + SB],
                                compare_op=mybir.AluOpType.is_ge,
                                fill=0.0, base=0,
                                pattern=[[0, 2], [1, SB]],
                                channel_multiplier=-1,
                            )
                        eg[(sb, g)] = e

                def emit_av(sb):
                    off = max(0, (sb - 4 * j) * SB)
                    es = {g: eg.pop((sb, g)) for g in range(NPAIR)}
                    for g in range(NPAIR):
                        for hh in range(2):
                            h = 2 * g + hh
                            nc.tensor.matmul(
                                av_ps[g][hh * 64:(hh + 1) * 64, off:],
                                lhsT=V[:, sb, h * 64:(h + 1) * 64],
                                rhs=es[g][:, hh, off:],
                                start=(sb == 0), stop=(sb == n_sb - 1),
                                skip_group_check=True,
                            )
                    for g in range(NPAIR):
                        for hh in range(2):
                            h = 2 * g + hh
                            nc.tensor.matmul(
                                sm_ps[32 * h:32 * h + 1, off:],
                                lhsT=ones1,
                                rhs=es[g][:, hh, off:],
                                start=(sb == 0), stop=(sb == n_sb - 1),
                                skip_group_check=True,
                                tile_position=(0, 32 * h),
                            )

                n_periods = n_sb + LAG
                n_pend = len(pending)
                popped = 0
                # Drain all pending work BEFORE the tail periods: anything
                # queued between this tile's last scores and its final AV
                # matmuls delays the av stop -> delays the output drains ->
                # (via the ScalarE queue) stalls the next tile's exps and
                # lets the HAM clock-gate re-throttle the PE.
                ramp = max(n_sb - LAG - 1, 4)
                for sb in range(n_periods):
                    if sb < n_sb:
                        emit_scores(sb)
                    want = min(n_pend, (n_pend * (sb + 1)) // ramp)
                    # the first pending items are the next xt load and the
                    # PREVIOUS tile's output drains; force them out in the
                    # first two periods -- a not-yet-drained av/sums PSUM
                    # bank would stall this tile's first AV matmul at the
                    # head of the PE queue, blocking everything behind it.
                    if sb == 0:
                        want = max(want, min(n_pend, 4))
                    elif sb == 1:
                        want = max(want, min(n_pend, 5))
                    while popped < want:
                        pending[popped]()
                        popped += 1
                    if sb >= LAG:
                        emit_av(sb - LAG)
                assert popped == n_pend

                # Output drains, returned as closures and emitted early in
                # the NEXT tile's attention (after its first exp) so the
                # ScalarE copies don't sit ahead of that tile's first exps
                # in the queue.
                last = j == nj - 1

                def drain_y(g):
                    y_sb = ysbp.tile([128, TT], F32, tag="ysb",
                                     name=f"y{j}_{g}")
                    # final tile: split across engines to shorten the tail
                    if last and g == 1:
                        nc.vector.tensor_copy(y_sb, av_ps[g])
                    else:
                        nc.scalar.copy(y_sb, av_ps[g])
                    nc.sync.dma_start(
                        out=yt[g * 128:(g + 1) * 128, j * TT:(j + 1) * TT],
                        in_=y_sb)

                def drain_sm():
                    # sums live on strided partitions {0,32,64,96}; engines
                    # can't compact partitions, so do ONE full-partition
                    # copy and let 4 DMAs pick out the used partitions.
                    sm_sb = ysbp.tile([128, TT], F32, tag="smsb",
                                      name=f"sm{j}")
                    nc.scalar.copy(sm_sb, sm_ps)
                    for h in range(HPC):
                        nc.sync.dma_start(
                            out=sm[j:j + 1, h:h + 1, :],
                            in_=sm_sb[32 * h:32 * h + 1, :])

                return [lambda: drain_y(0), lambda: drain_y(1), drain_sm]

            ld0_8, ld0_16, ops0 = proj_closures(0)
            ld0_8()
            nc.sync.dma_start(
                out=wq_sb, in_=wq.rearrange("(k p) d -> p k d", p=128))
            sm_ps_warm = smps.tile([128, TT], F32, tag="smps", name="smwarm")
            # PE warm-up: dependency-free tiny matmuls during the DMA fill
            # so the HAM clock-gate is at 8/8 when the first projection
            # matmul issues (cold K=4/8 costs 2x). Writes scratch into the
            # sums bank; attention later overwrites it via start=True.
            for _ in range(240):
                nc.tensor.matmul(sm_ps_warm[0:1, 0:1], lhsT=ones1,
                                 rhs=ones1, start=True, stop=True,
                                 skip_group_check=True)
            # DMA ring order: fp8 xt (done), wq, wk, THEN the bf16 xt and
            # wv -- q/k proj (which gates the fill) only waits on the first
            # 2.5MB; the v path streams in behind it.
            nc.sync.dma_start(
                out=wk_sb, in_=wk.rearrange("(k p) d -> p k d", p=128))
            ops0[0]()
            ops0[1]()
            ld0_16()
            ops0[2]()
            ops0[3]()
            nc.sync.dma_start(
                out=wv_sb, in_=wv.rearrange("(k p) d -> p k d", p=128))
            ops0[4]()
            ops0[5]()
            drains = []
            for j in range(nj):
                if j + 1 < nj:
                    ld8n, ld16n, opsn = proj_closures(j + 1)
                    nxt = [ld8n, ld16n] + drains + opsn
                else:
                    nxt = list(drains)
                drains = emit_attention(j, nxt)
            for dr in drains:
                dr()

    nc.compile()
    return nc


_CACHE = {}


def _get_runner():
    if "run" in _CACHE:
        return _CACHE["run"]

    import jax
    from jax.experimental.shard_map import shard_map
    from jax.sharding import Mesh, PartitionSpec
    from concourse import bass2jax
    from concourse.bass2jax import _bass_exec_p, install_neuronx_cc_hook

    nc = build_nc()
    install_neuronx_cc_hook()

    partition_name = (nc.partition_id_tensor.name
                      if nc.partition_id_tensor else None)
    in_names, out_names, out_avals, zero_outs = [], [], [], []
    for alloc in nc.m.functions[0].allocations:
        if not isinstance(alloc, mybir.MemoryLocationSet):
            continue
        name = alloc.memorylocations[0].name
        if alloc.kind == "ExternalInput":
            if name != partition_name:
                in_names.append(name)
        elif alloc.kind == "ExternalOutput":
            out_names.append(name)
            shape = tuple(alloc.tensor_shape)
            dtype = mybir.dt.np(alloc.dtype)
            out_avals.append(jax.core.ShapedArray(shape, dtype))
            zero_outs.append(np.zeros(shape, dtype))
    n_params = len(in_names)
    n_outs = len(out_avals)
    all_names = in_names + out_names
    if partition_name is not None:
        all_names = all_names + [partition_name]
    donate = tuple(range(n_params, n_params + n_outs))

    def _body(*args):
        operands = list(args)
        if partition_name is not None:
            operands.append(bass2jax.partition_id_tensor())
        outs = _bass_exec_p.bind(
            *operands,
            out_avals=tuple(out_avals),
            in_names=tuple(all_names),
            out_names=tuple(out_names),
            lowering_input_output_aliases=(),
            sim_require_finite=True,
            sim_require_nnan=True,
            nc=nc,
        )
        return tuple(outs)

    devices = jax.devices()[:NCORES]
    mesh = Mesh(np.asarray(devices), ("core",))
    in_specs = (PartitionSpec("core"),) * (n_params + n_outs)
    out_specs = (PartitionSpec("core"),) * n_outs
    sharded = jax.jit(
        shard_map(_body, mesh=mesh, in_specs=in_specs, out_specs=out_specs,
                  check_rep=False),
        donate_argnums=donate, keep_unused=True,
    )

    runner = {
        "nc": nc,
        "all_names": all_names,
        "sharded": sharded,
        "in_names": in_names,
        "out_names": out_names,
        "out_avals": out_avals,
        "zero_outs": zero_outs,
    }
    _CACHE["run"] = runner
    return runner


def _shard_inputs(x, Wq, Wk, Wv):
    """Per-core input dicts. Host-side layout prep only."""
    bf = mybir.dt.np(BF16)
    f8 = mybir.dt.np(F8)
    maps = []
    for c in range(NCORES):
        b, hg = divmod(c, 4)
        hs = list(range(HPC * hg, HPC * hg + HPC))
        xtb = np.ascontiguousarray(np.transpose(x[b]))  # [C, T]
        # DR scores layout: col (o*128 + 32h + r) = W[h][:, 32o + r]
        def perm_dr(W):
            wc = np.stack([W[h] for h in hs], 0)          # [4, C, 64]
            wc = wc.reshape(HPC, C, 2, 32)
            return np.ascontiguousarray(
                wc.transpose(1, 2, 0, 3).reshape(C, HPC * D))
        wq2 = (perm_dr(Wq) * WS).astype(f8)
        wk2 = (perm_dr(Wk) * WS).astype(f8)
        wv2 = np.ascontiguousarray(
            np.concatenate([Wv[h] for h in hs], axis=1).astype(bf))
        maps.append({"xt": xtb.astype(bf), "xt8": xtb.astype(f8),
                     "wq": wq2, "wk": wk2, "wv": wv2})
    return maps


def run_sharded(in_maps):
    """Run the 8-core NEFF once; returns list of per-core output dicts."""
    r = _get_runner()
    concat_in = [
        np.concatenate([in_maps[c][name] for c in range(NCORES)], axis=0)
        for name in r["in_names"]
    ]
    concat_zeros = [
        np.zeros((NCORES * z.shape[0], *z.shape[1:]), z.dtype)
        for z in r["zero_outs"]
    ]
    out_arrs = r["sharded"](*concat_in, *concat_zeros)
    return [
        {
            name: np.asarray(out_arrs[i]).reshape(
                NCORES, *r["out_avals"][i].shape)[c]
            for i, name in enumerate(r["out_names"])
        }
        for c in range(NCORES)
    ]


def kernel(x, Wq, Wk, Wv):
    x = np.asarray(x, dtype=np.float32)
    Wq = np.asarray(Wq, dtype=np.float32)
    Wk = np.asarray(Wk, dtype=np.float32)
    Wv = np.asarray(Wv, dtype=np.float32)
    in_maps = _shard_inputs(x, Wq, Wk, Wv)
    results = run_sharded(in_maps)
    outs = []
    for b in range(B):
        parts = []
        for hg in range(4):
            r = results[b * 4 + hg]
            ytc = np.asarray(r["yt"], dtype=np.float32)   # [256, T]
            smc = np.asarray(r["sm"], dtype=np.float32)   # [nj, 4, TT]
            smc = smc.transpose(1, 0, 2).reshape(HPC, T)  # [4, T]
            yn = ytc.reshape(HPC, D, T) / smc[:, None, :]
            parts.append(yn.reshape(HPC * D, T).T)        # [T, 256]
        outs.append(np.concatenate(parts, axis=1))        # [T, 1024]
    return np.ascontiguousarray(np.stack(outs)).astype(np.float32)


# revision 12
# speedup vs baseline: 1.2706x; 1.2706x over previous
"""Multi-head causal attention (B=2, T=2048, C=1024, H=16, D=64) on 8 TRN2 cores.

Sharding: core c = (batch b = c//4, head-group hg = c%4): 4 heads of one batch
per core. Host concatenates heads / batches and normalizes (divide by softmax
sums) + transposes on the way out.

Per-core dataflow (all matmuls accumulate f32 in PSUM):
  1. q/k projections fp8 x fp8 DoubleRow: W chunk stationary [128c, 2, 128],
     x8^T moving [128c, 2, 512] -> Q~/K~ = x.Wq (un-downscaled) drained as
     fp8e4 into the DR scores layout: QT8/KT8[128p, 2, T] where partition
     p = 32h + (d&31) and the middle index is d>>5 (host permutes Wq/Wk
     columns so the projection writes this layout directly).
  2. v projection TRANSPOSED: x^T chunk stationary [128c, 128t] bf16, Wv
     moving [128c, 256hd] -> V[t, hd] directly (no PE transposes).
  3. Scores S^T[s, 4h, t]: fp8 DoubleRow, ALL FOUR heads concurrent via
     row-tiling (each head owns PE rows 32h..32h+31 with its d=64
     contraction 2-packed). One wave of 4 MMs per s-block. The 1/sqrt(C)
     softmax scale is folded into the exp (free affine / Schraudolph mult).
     Columns below the causal diagonal never computed (off trim).
  4. exp: per (s-block, head-pair) block. Short softmax rows (t<128) live in
     s-block 0 which is exact on ScalarE; other blocks split Scalar/DVE
     (DVE = 1-op Schraudolph: E_bf16bits = int16(S*(128/ln2)/32 + b)),
     balanced so both engines carry ~equal evacuation load. Diagonal 128x128
     gets its causal triangle zeroed by GpSimd affine_select (GpSimd is
     otherwise idle; keeps DVE free).
  5. AV: V[s,d] stationary per head, E^T moving; two heads of a pair run
     concurrently via col-tiling -> av^T[d-pair, t], already y^T layout.
     Softmax sums via 4 concurrent M=1 col-tiled matmuls into partitions
     {0,32,64,96}.
  6. Drains: y = av PSUM -> SBUF copy -> DMA. sums: ONE full-partition copy
     PSUM->SBUF then 4 single-partition DMAs (engine APs can't compact
     strided partitions, DMAs can address them).
  7. Host: y = (yt / sums) per head, transpose, concat.

Schedule: fused streaming pipeline per t-tile; next tile's projection closures
drain into the current tile's attention periods; AV lags scores by 2 s-blocks.
PSUM: scores 2x2 banks + av 2x1 + sums 1 + proj 1 = exactly 8 banks.
"""

import numpy as np

import concourse.mybir as mybir
import concourse.tile as tile
from concourse import bacc

B, T, C, H, D = 2, 2048, 1024, 16, 64
HPC = 4          # heads per core
NPAIR = 2        # head pairs per core
NCORES = 8
TT = 512         # t-tile
SB = 128         # s-block
NCH = C // 128   # contraction chunks
NTB = T // 128   # t-blocks (v-proj granularity)
F32 = mybir.dt.float32
BF16 = mybir.dt.bfloat16
F8 = mybir.dt.float8e4
I16 = mybir.dt.int16
WS = 32.0        # fp8 weight pre-scale (W*32 fits e4m3; drains divide it out)
SCALE = float(C) ** -0.5   # softmax scale, folded into exp

# exp(x) ~= bitcast_bf16(int16(x * 128/ln2 + (127*128 - c))): Schraudolph in
# bf16 bits. Scaled scores are in [-1, 1] so the int is ~16065..16435: no
# overflow, no sign issues. c calibrated offline.
SCH_K = float(2.0 ** 7 / np.log(2.0))
SCH_B = float(127 * 128 - 5.0)


# Scores/exp run at half-t-tile granularity (HB=256 columns): each S PSUM
# tile [128, 2, HB] f32 is exactly ONE bank, giving 4 independently-recycled
# slots in the same 4 banks. The scores(sb+1) matmul waits on exp(sb) of the
# same slot (PSUM WAR), so halving the exp block size halves the attention
# pipeline's critical-path period. Each exp block runs WHOLE on one engine
# (no cross-engine rendezvous per block); a greedy balancer assigns engines.
HB = 512
# per-instruction overhead (cycles @1.2GHz) used by the engine balancer
OV_SCALAR = 352.0
OV_DVE = 97.0


def build_nc():
    nj = T // TT
    nc = bacc.Bacc("TRN2", target_bir_lowering=False, debug=False)
    xt = nc.dram_tensor("xt", [C, T], BF16, kind="ExternalInput")
    xt8 = nc.dram_tensor("xt8", [C, T], F8, kind="ExternalInput")
    wq = nc.dram_tensor("wq", [C, HPC * D], F8, kind="ExternalInput")
    wk = nc.dram_tensor("wk", [C, HPC * D], F8, kind="ExternalInput")
    wv = nc.dram_tensor("wv", [C, HPC * D], BF16, kind="ExternalInput")
    yt = nc.dram_tensor("yt", [HPC * D, T], F32, kind="ExternalOutput")
    sm = nc.dram_tensor("sm", [nj, HPC, TT], F32, kind="ExternalOutput")

    with tile.TileContext(nc) as tc:
        with (
            tc.tile_pool(name="wpool", bufs=1) as wpool,
            tc.tile_pool(name="xtp", bufs=2) as xtp,
            tc.tile_pool(name="epool", bufs=8) as epool,
            tc.tile_pool(name="ysb", bufs=4) as ysbp,
            tc.tile_pool(name="mixps", bufs=1, space="PSUM") as mixps,
            tc.tile_pool(name="spsum", bufs=2, space="PSUM") as spsum,
            tc.tile_pool(name="avps", bufs=2, space="PSUM") as avps,
            tc.tile_pool(name="smps", bufs=1, space="PSUM") as smps,
        ):
            # greedy engine-load balancer (emission-time, deterministic)
            ecost = {"s": 0.0, "v": 0.0}
            eprev = {}

            def pick_engine(stream_cyc, slot=None):
                cs = ecost["s"] + stream_cyc + OV_SCALAR
                cv = ecost["v"] + stream_cyc + OV_DVE
                # prefer alternating engines along a slot's release chain
                if slot is not None and slot in eprev:
                    if eprev[slot] == "s":
                        cs += 200.0
                    else:
                        cv += 200.0
                eng = "s" if cs <= cv else "v"
                ecost[eng] = (cs if eng == "s" else cv)
                if slot is not None:
                    eprev[slot] = eng
                return eng
            ones1 = wpool.tile([128, 1], BF16)
            nc.vector.memset(ones1, 1.0)

            wq_sb = wpool.tile([128, NCH, HPC * D], F8, tag="wq")
            wk_sb = wpool.tile([128, NCH, HPC * D], F8, tag="wk")
            wv_sb = wpool.tile([128, NCH, HPC * D], BF16, tag="wv")
            w_sb = {"q": wq_sb, "k": wk_sb}

            # DR scores layout: partition 32h + (d&31), middle dim d>>5
            QT8 = wpool.tile([128, 2, T], F8, tag="qt8")
            KT8 = wpool.tile([128, 2, T], F8, tag="kt8")
            qk_dst = {"q": QT8, "k": KT8}
            V = wpool.tile([128, NTB, HPC * D], BF16, tag="v")

            def proj_closures(j):
                state = {}

                def do_load8():
                    # fp8 copy first (q/k proj gates the pipeline fill);
                    # chunk-pair granularity so the first matmuls only wait
                    # for the first 2 contraction chunks, not the full tile.
                    x8r = xt8.rearrange("(k p) t -> p k t", p=128)
                    xt8_sb = xtp.tile([128, NCH, TT], F8, tag="xts8",
                                      name=f"xts8_{j}")
                    for kk in range(0, NCH, 2):
                        nc.sync.dma_start(
                            out=xt8_sb[:, kk:kk + 2, :],
                            in_=x8r[:, kk:kk + 2, j * TT:(j + 1) * TT])
                    state["xt8"] = xt8_sb

                def do_load16():
                    xr = xt.rearrange("(k p) t -> p k t", p=128)
                    xt_sb = xtp.tile([128, NCH, TT], BF16, tag="xts",
                                     name=f"xts{j}")
                    for kk in range(0, NCH, 4):
                        nc.sync.dma_start(
                            out=xt_sb[:, kk:kk + 4, :],
                            in_=xr[:, kk:kk + 4, j * TT:(j + 1) * TT])
                    state["xt"] = xt_sb

                def do_qk(name, half):
                    # fp8e4 DoubleRow: two contraction chunks per matmul
                    pp = mixps.tile([128, TT], F32, tag="mix",
                                    name=f"pp_{name}{half}_{j}")
                    for kk in range(0, NCH, 2):
                        nc.tensor.matmul(
                            pp,
                            lhsT=w_sb[name][:, kk:kk + 2,
                                            half * 128:(half + 1) * 128],
                            rhs=state["xt8"][:, kk:kk + 2, :],
                            start=(kk == 0), stop=(kk == NCH - 2),
                            skip_group_check=True,
                            perf_mode=mybir.MatmulPerfMode.DoubleRow,
                        )
                    ecost["v"] += TT + OV_DVE
                    nc.vector.tensor_scalar(
                        qk_dst[name][:, half, j * TT:(j + 1) * TT], pp,
                        1.0 / WS, None, mybir.AluOpType.mult)

                def do_v(tb2):
                    vp = mixps.tile([128, TT], F32, tag="mix",
                                    name=f"vp{tb2}_{j}")
                    for half in range(2):
                        tl = tb2 * 2 + half
                        for kk in range(NCH):
                            nc.tensor.matmul(
                                vp[:, half * 256:(half + 1) * 256],
                                lhsT=state["xt"][:, kk, tl * 128:(tl + 1) * 128],
                                rhs=wv_sb[:, kk, :],
                                start=(kk == 0), stop=(kk == NCH - 1),
                                skip_group_check=True,
                            )
                    gtb = j * 4 + tb2 * 2
                    ecost["v"] += TT + OV_DVE
                    nc.vector.tensor_copy(
                        V[:, gtb:gtb + 2, :],
                        vp.rearrange("p (a b) -> p a b", a=2))

                ops = [lambda: do_qk("q", 0), lambda: do_qk("q", 1),
                       lambda: do_qk("k", 0), lambda: do_qk("k", 1),
                       lambda: do_v(0), lambda: do_v(1)]
                return do_load8, do_load16, ops

            def emit_attention(j, pending):
                n_sb = 4 * (j + 1)
                av_ps = {g: avps.tile([128, TT], F32, tag="avps",
                                      name=f"av{j}_{g}")
                         for g in range(NPAIR)}
                sm_ps = smps.tile([128, TT], F32, tag="smps", name=f"smps{j}")
                eg = {}
                LAG = 2

                def emit_scores(sb):
                    off = max(0, (sb - 4 * j) * SB)
                    S = {}
                    for th in range(TT // HB):
                        if max(off, th * HB) >= (th + 1) * HB:
                            continue
                        for g in range(NPAIR):
                            S[(g, th)] = spsum.tile(
                                [128, 2, HB], F32, tag="spsum",
                                name=f"s{j}_{sb}_{g}_{th}")
                    # scores waves per t-half: 4 heads concurrent (row-tiled
                    # DoubleRow, each head's d=64 packed 2/cell in 32 rows)
                    for th in range(TT // HB):
                        lo = max(off, th * HB)
                        hi = (th + 1) * HB
                        if lo >= hi:
                            continue
                        for h in range(HPC):
                            g, hh = h // 2, h % 2
                            hp = slice(32 * h, 32 * h + 32)
                            nc.tensor.matmul(
                                S[(g, th)][:, hh, lo - th * HB:],
                                lhsT=KT8[hp, :, sb * SB:(sb + 1) * SB],
                                rhs=QT8[hp, :, j * TT + lo:j * TT + hi],
                                start=True, stop=True,
                                perf_mode=mybir.MatmulPerfMode.DoubleRow,
                                tile_position=(32 * h, 0),
                            )
                    e = {g: epool.tile([128, 2, TT], BF16, tag="e",
                                       name=f"e{j}_{sb}_{g}")
                         for g in range(NPAIR)}
                    th_diag = off // HB  # t-half containing the diagonal
                    for th in range(TT // HB):
                        lo = max(off, th * HB)
                        hi = (th + 1) * HB
                        if lo >= hi:
                            continue
                        w = hi - lo
                        for g in range(NPAIR):
                            exact = j == 0 and sb == 0 and th == 0
                            if exact:
                                eng = "s"
                                ecost["s"] += 2 * w + OV_SCALAR
                            else:
                                eng = pick_engine(2 * w, slot=(g, th))
                            if eng == "s":
                                nc.scalar.activation(
                                    out=e[g][:, :, lo:hi],
                                    in_=S[(g, th)][:, :, lo - th * HB:],
                                    func=mybir.ActivationFunctionType.Exp,
                                    scale=SCALE)
                            else:
                                nc.vector.tensor_scalar(
                                    e[g].bitcast(I16)[:, :, lo:hi],
                                    S[(g, th)][:, :, lo - th * HB:],
                                    SCH_K * SCALE, SCH_B,
                                    mybir.AluOpType.mult,
                                    mybir.AluOpType.add)
                            if sb >= 4 * j and th == th_diag:
                                # diagonal block: causal triangle on GpSimd
                                nc.gpsimd.affine_select(
                                    out=e[g][:, :, off:off + SB],
                                    in_=e[g][:, :, off:off + SB],
                                    compare_op=mybir.AluOpType.is_ge,
                                    fill=0.0, base=0,
                                    pattern=[[0, 2], [1, SB]],
                                    channel_multiplier=-1,
                                )
                    for g in range(NPAIR):
                        eg[(sb, g)] = e[g]

                def emit_av(sb):
                    off = max(0, (sb - 4 * j) * SB)
                    es = {g: eg.pop((sb, g)) for g in range(NPAIR)}
                    for g in range(NPAIR):
                        for hh in range(2):
                            h = 2 * g + hh
                            nc.tensor.matmul(
                                av_ps[g][hh * 64:(hh + 1) * 64, off:],
                                lhsT=V[:, sb, h * 64:(h + 1) * 64],
                                rhs=es[g][:, hh, off:],
                                start=(sb == 0), stop=(sb == n_sb - 1),
                                skip_group_check=True,
                            )
                    for g in range(NPAIR):
                        for hh in range(2):
                            h = 2 * g + hh
                            nc.tensor.matmul(
                                sm_ps[32 * h:32 * h + 1, off:],
                                lhsT=ones1,
                                rhs=es[g][:, hh, off:],
                                start=(sb == 0), stop=(sb == n_sb - 1),
                                skip_group_check=True,
                                tile_position=(0, 32 * h),
                            )

                n_periods = n_sb + LAG
                n_pend = len(pending)
                popped = 0
                # Drain all pending work BEFORE the tail periods: anything
                # queued between this tile's last scores and its final AV
                # matmuls delays the av stop -> delays the output drains ->
                # (via the ScalarE queue) stalls the next tile's exps and
                # lets the HAM clock-gate re-throttle the PE.
                ramp = max(n_sb - LAG - 1, 4)
                for sb in range(n_periods):
                    if sb < n_sb:
                        emit_scores(sb)
                    want = min(n_pend, (n_pend * (sb + 1)) // ramp)
                    # the first pending items are the next xt load and the
                    # PREVIOUS tile's output drains; force them out in the
                    # first two periods -- a not-yet-drained av/sums PSUM
                    # bank would stall this tile's first AV matmul at the
                    # head of the PE queue, blocking everything behind it.
                    if sb == 0:
                        want = max(want, min(n_pend, 4))
                    elif sb == 1:
                        want = max(want, min(n_pend, 5))
                    while popped < want:
                        pending[popped]()
                        popped += 1
                    if sb >= LAG:
                        emit_av(sb - LAG)
                assert popped == n_pend

                # Output drains, returned as closures and emitted early in
                # the NEXT tile's attention (after its first exp) so the
                # ScalarE copies don't sit ahead of that tile's first exps
                # in the queue.
                last = j == nj - 1

                def drain_y(g):
                    y_sb = ysbp.tile([128, TT], F32, tag="ysb",
                                     name=f"y{j}_{g}")
                    # final tile: split across engines to shorten the tail
                    if last and g == 1:
                        ecost["v"] += TT + OV_DVE
                        nc.vector.tensor_copy(y_sb, av_ps[g])
                    else:
                        ecost["s"] += TT + OV_SCALAR
                        nc.scalar.copy(y_sb, av_ps[g])
                    nc.sync.dma_start(
                        out=yt[g * 128:(g + 1) * 128, j * TT:(j + 1) * TT],
                        in_=y_sb)

                def drain_sm():
                    # sums live on strided partitions {0,32,64,96}; engines
                    # can't compact partitions, so do ONE full-partition
                    # copy and let 4 DMAs pick out the used partitions.
                    sm_sb = ysbp.tile([128, TT], F32, tag="smsb",
                                      name=f"sm{j}")
                    ecost["s"] += TT + OV_SCALAR
                    nc.scalar.copy(sm_sb, sm_ps)
                    for h in range(HPC):
                        nc.sync.dma_start(
                            out=sm[j:j + 1, h:h + 1, :],
                            in_=sm_sb[32 * h:32 * h + 1, :])

                return [lambda: drain_y(0), lambda: drain_y(1), drain_sm]

            ld0_8, ld0_16, ops0 = proj_closures(0)
            ld0_8()
            nc.sync.dma_start(
                out=wq_sb, in_=wq.rearrange("(k p) d -> p k d", p=128))
            sm_ps_warm = smps.tile([128, TT], F32, tag="smps", name="smwarm")
            # PE warm-up: dependency-free tiny matmuls during the DMA fill
            # so the HAM clock-gate is at 8/8 when the first projection
            # matmul issues (cold K=4/8 costs 2x). Writes scratch into the
            # sums bank; attention later overwrites it via start=True.
            for _ in range(240):
                nc.tensor.matmul(sm_ps_warm[0:1, 0:1], lhsT=ones1,
                                 rhs=ones1, start=True, stop=True,
                                 skip_group_check=True)
            # DMA ring order: fp8 xt (done), wq, wk, THEN the bf16 xt and
            # wv -- q/k proj (which gates the fill) only waits on the first
            # 2.5MB; the v path streams in behind it.
            nc.sync.dma_start(
                out=wk_sb, in_=wk.rearrange("(k p) d -> p k d", p=128))
            ops0[0]()
            ops0[1]()
            ld0_16()
            ops0[2]()
            ops0[3]()
            nc.sync.dma_start(
                out=wv_sb, in_=wv.rearrange("(k p) d -> p k d", p=128))
            ops0[4]()
            ops0[5]()
            drains = []
            for j in range(nj):
                if j + 1 < nj:
                    ld8n, ld16n, opsn = proj_closures(j + 1)
                    nxt = [ld8n, ld16n] + drains + opsn
                else:
                    nxt = list(drains)
                drains = emit_attention(j, nxt)
            for dr in drains:
                dr()

    nc.compile()
    return nc


_CACHE = {}


def _get_runner():
    if "run" in _CACHE:
        return _CACHE["run"]

    import jax
    from jax.experimental.shard_map import shard_map
    from jax.sharding import Mesh, PartitionSpec
    from concourse import bass2jax
    from concourse.bass2jax import _bass_exec_p, install_neuronx_cc_hook

    nc = build_nc()
    install_neuronx_cc_hook()

    partition_name = (nc.partition_id_tensor.name
                      if nc.partition_id_tensor else None)
    in_names, out_names, out_avals, zero_outs = [], [], [], []
    for alloc in nc.m.functions[0].allocations:
        if not isinstance(alloc, mybir.MemoryLocationSet):
            continue
        name = alloc.memorylocations[0].name
        if alloc.kind == "ExternalInput":
            if name != partition_name:
                in_names.append(name)
        elif alloc.kind == "ExternalOutput":
            out_names.append(name)
            shape = tuple(alloc.tensor_shape)
            dtype = mybir.dt.np(alloc.dtype)
            out_avals.append(jax.core.ShapedArray(shape, dtype))
            zero_outs.append(np.zeros(shape, dtype))
    n_params = len(in_names)
    n_outs = len(out_avals)
    all_names = in_names + out_names
    if partition_name is not None:
        all_names = all_names + [partition_name]
    donate = tuple(range(n_params, n_params + n_outs))

    def _body(*args):
        operands = list(args)
        if partition_name is not None:
            operands.append(bass2jax.partition_id_tensor())
        outs = _bass_exec_p.bind(
            *operands,
            out_avals=tuple(out_avals),
            in_names=tuple(all_names),
            out_names=tuple(out_names),
            lowering_input_output_aliases=(),
            sim_require_finite=True,
            sim_require_nnan=True,
            nc=nc,
        )
        return tuple(outs)

    devices = jax.devices()[:NCORES]
    mesh = Mesh(np.asarray(devices), ("core",))
    in_specs = (PartitionSpec("core"),) * (n_params + n_outs)
    out_specs = (PartitionSpec("core"),) * n_outs
    sharded = jax.jit(
        shard_map(_body, mesh=mesh, in_specs=in_specs, out_specs=out_specs,
                  check_rep=False),
        donate_argnums=donate, keep_unused=True,
    )

    runner = {
        "nc": nc,
        "all_names": all_names,
        "sharded": sharded,
        "in_names": in_names,
        "out_names": out_names,
        "out_avals": out_avals,
        "zero_outs": zero_outs,
    }
    _CACHE["run"] = runner
    return runner


def _shard_inputs(x, Wq, Wk, Wv):
    """Per-core input dicts. Host-side layout prep only."""
    bf = mybir.dt.np(BF16)
    f8 = mybir.dt.np(F8)
    maps = []
    for c in range(NCORES):
        b, hg = divmod(c, 4)
        hs = list(range(HPC * hg, HPC * hg + HPC))
        xtb = np.ascontiguousarray(np.transpose(x[b]))  # [C, T]
        # DR scores layout: col (o*128 + 32h + r) = W[h][:, 32o + r]
        def perm_dr(W):
            wc = np.stack([W[h] for h in hs], 0)          # [4, C, 64]
            wc = wc.reshape(HPC, C, 2, 32)
            return np.ascontiguousarray(
                wc.transpose(1, 2, 0, 3).reshape(C, HPC * D))
        wq2 = (perm_dr(Wq) * WS).astype(f8)
        wk2 = (perm_dr(Wk) * WS).astype(f8)
        wv2 = np.ascontiguousarray(
            np.concatenate([Wv[h] for h in hs], axis=1).astype(bf))
        maps.append({"xt": xtb.astype(bf), "xt8": xtb.astype(f8),
                     "wq": wq2, "wk": wk2, "wv": wv2})
    return maps


def run_sharded(in_maps):
    """Run the 8-core NEFF once; returns list of per-core output dicts."""
    r = _get_runner()
    concat_in = [
        np.concatenate([in_maps[c][name] for c in range(NCORES)], axis=0)
        for name in r["in_names"]
    ]
    concat_zeros = [
        np.zeros((NCORES * z.shape[0], *z.shape[1:]), z.dtype)
        for z in r["zero_outs"]
    ]
    out_arrs = r["sharded"](*concat_in, *concat_zeros)
    return [
        {
            name: np.asarray(out_arrs[i]).reshape(
                NCORES, *r["out_avals"][i].shape)[c]
            for i, name in enumerate(r["out_names"])
        }
        for c in range(NCORES)
    ]


def kernel(x, Wq, Wk, Wv):
    x = np.asarray(x, dtype=np.float32)
    Wq = np.asarray(Wq, dtype=np.float32)
    Wk = np.asarray(Wk, dtype=np.float32)
    Wv = np.asarray(Wv, dtype=np.float32)
    in_maps = _shard_inputs(x, Wq, Wk, Wv)
    results = run_sharded(in_maps)
    outs = []
    for b in range(B):
        parts = []
        for hg in range(4):
            r = results[b * 4 + hg]
            ytc = np.asarray(r["yt"], dtype=np.float32)   # [256, T]
            smc = np.asarray(r["sm"], dtype=np.float32)   # [nj, 4, TT]
            smc = smc.transpose(1, 0, 2).reshape(HPC, T)  # [4, T]
            yn = ytc.reshape(HPC, D, T) / smc[:, None, :]
            parts.append(yn.reshape(HPC * D, T).T)        # [T, 256]
        outs.append(np.concatenate(parts, axis=1))        # [T, 1024]
    return np.ascontiguousarray(np.stack(outs)).astype(np.float32)


# revision 19
# speedup vs baseline: 1.3015x; 1.0243x over previous
"""Multi-head causal attention (B=2, T=2048, C=1024, H=16, D=64) on 8 TRN2 cores.

Sharding: core c = (batch b = c//4, head-group hg = c%4): 4 heads of one batch
per core (halves x DMA vs replicating both batches). Host concatenates heads /
batches and normalizes (divide by softmax sums) + transposes on the way out.

Per-core dataflow, all matmuls bf16 x bf16 -> f32 PSUM (fast FWL weight loads;
LDWEIGHTS hides under the previous matmul, unlike f32r self-loading):
  1. q/k projections: W chunk stationary [128c, 128hd], x^T moving [128c, 512t]
     -> Q^T/K^T [hd, t] bf16, head pair g on partitions (h even: 0-63,
     h odd: 64-127). Scale 1/sqrt(C) folded into Wq on host.
  2. v projection TRANSPOSED: x^T chunk stationary [128c, 128t], Wv moving
     [128c, 256hd] -> V[t, hd] directly (no PE transposes at all).
  3. Scores S^T[s, 2, t] per (s-block, pair): K^T stationary, Q^T moving; the
     two heads of a pair run CONCURRENTLY via row-tiling (contraction d=64 ->
     tile_position (0,0)/(64,0) auto-derived from base partitions). Columns
     below the causal diagonal never computed (off trim).
  4. exp: s-block 0 (plus a tunable share) on ScalarE (exact); the rest on DVE
     via a 1-op Schraudolph: E_bf16bits = int16(S * 128/ln2 + (127*128-c)).
     Short softmax rows (t<128) live entirely in s-block 0, so keeping that
     block exact pins max rel err at the bf16 floor (~3e-3, validated offline).
     Diagonal 128x128 gets a multiplicative 0/1 triangle (DVE).
  5. AV: V[s,d] stationary per head, E^T moving; the two heads of a pair run
     CONCURRENTLY via col-tiling (output partitions 0-63 / 64-127 of one PSUM
     bank, tile_position (0,0)/(0,64) auto). Output is av^T[d-pair, t] --
     already in the y^T layout, no transposes. Softmax sums via 4 concurrent
     M=1 col-tiled matmuls (ones stationary) into partitions {0,32,64,96}.
  6. Host: y = (yt / sums) per head, transpose, concat.

Schedule: fused streaming pipeline per t-tile; next tile's projection closures
drain into the current tile's attention periods; AV lags scores by 2 s-blocks.
PSUM: scores 2x2 banks + av 2x1 + sums 1 + proj 1 = exactly 8 banks.
"""

import numpy as np

import concourse.mybir as mybir
import concourse.tile as tile
from concourse import bacc

B, T, C, H, D = 2, 2048, 1024, 16, 64
HPC = 4          # heads per core
NPAIR = 2        # head pairs per core
NCORES = 8
TT = 512         # t-tile
SB = 128         # s-block
NCH = C // 128   # contraction chunks
NTB = T // 128   # t-blocks (v-proj granularity)
F32 = mybir.dt.float32
BF16 = mybir.dt.bfloat16
F8 = mybir.dt.float8e4
I16 = mybir.dt.int16
WS = 32.0        # fp8 weight pre-scale (W*32 fits e4m3; drains divide it out)

# exp(x) ~= bitcast_bf16(int16(x * 128/ln2 + (127*128 - c))): Schraudolph in
# bf16 bits. Scores are in [-1, 1] so the int is ~16065..16435: no overflow,
# no sign issues. c calibrated offline; end-to-end error is insensitive to c
# and to round-vs-floor convert semantics (validated in numpy).
SCH_K = float(2.0 ** 7 / np.log(2.0))
SCH_B = float(127 * 128 - 5.0)

# exp engine assignment: s-block 0 goes to ScalarE for BOTH pairs (exact exp
# protects short softmax rows); for sb>0 the two pairs split across ScalarE
# and DVE (alternating by sb so each head sees a ~50/50 exact/approx mix and
# each S-buffer's release chain alternates engines -> no single-engine
# backlog stalls the scores matmuls on the PSUM WAR).


def build_nc():
    nj = T // TT
    nc = bacc.Bacc("TRN2", target_bir_lowering=False, debug=False)
    xt = nc.dram_tensor("xt", [C, T], BF16, kind="ExternalInput")
    xt8 = nc.dram_tensor("xt8", [C, T], F8, kind="ExternalInput")
    wq = nc.dram_tensor("wq", [C, HPC * D], F8, kind="ExternalInput")
    wk = nc.dram_tensor("wk", [C, HPC * D], F8, kind="ExternalInput")
    wv = nc.dram_tensor("wv", [C, HPC * D], BF16, kind="ExternalInput")
    yt = nc.dram_tensor("yt", [HPC * D, T], F32, kind="ExternalOutput")
    sm = nc.dram_tensor("sm", [nj, HPC, TT], F32, kind="ExternalOutput")

    with tile.TileContext(nc) as tc:
        with (
            tc.tile_pool(name="wpool", bufs=1) as wpool,
            tc.tile_pool(name="xtp", bufs=2) as xtp,
            tc.tile_pool(name="epool", bufs=8) as epool,
            tc.tile_pool(name="ysb", bufs=4) as ysbp,
            tc.tile_pool(name="mixps", bufs=1, space="PSUM") as mixps,
            tc.tile_pool(name="spsum", bufs=2, space="PSUM") as spsum,
            tc.tile_pool(name="avps", bufs=2, space="PSUM") as avps,
            tc.tile_pool(name="smps", bufs=1, space="PSUM") as smps,
        ):
            # greedy engine-load balancer (emission-time, deterministic)
            ecost = {"s": 0.0, "v": 0.0}
            eprev = {}
            OV_S, OV_V = 352.0, 97.0

            def pick_engine(stream_cyc, slot=None):
                cs = ecost["s"] + stream_cyc + OV_S
                cv = ecost["v"] + stream_cyc + OV_V
                # prefer alternating engines along a slot's release chain
                if slot is not None and slot in eprev:
                    if eprev[slot] == "s":
                        cs += 200.0
                    else:
                        cv += 200.0
                eng = "s" if cs <= cv else "v"
                ecost[eng] = (cs if eng == "s" else cv)
                if slot is not None:
                    eprev[slot] = eng
                return eng
            ones1 = wpool.tile([128, 1], BF16)
            nc.vector.memset(ones1, 1.0)

            wq_sb = wpool.tile([128, NCH, HPC * D], F8, tag="wq")
            wk_sb = wpool.tile([128, NCH, HPC * D], F8, tag="wk")
            wv_sb = wpool.tile([128, NCH, HPC * D], BF16, tag="wv")
            w_sb = {"q": wq_sb, "k": wk_sb}

            QT = wpool.tile([128, NPAIR, T], BF16, tag="qt")
            KT = wpool.tile([128, NPAIR, T], BF16, tag="kt")
            V = wpool.tile([128, NTB, HPC * D], BF16, tag="v")

            def proj_closures(j):
                state = {}

                def do_load8():
                    # fp8 copy first (q/k proj gates the pipeline fill);
                    # chunk-pair granularity so the first matmuls only wait
                    # for the first 2 contraction chunks, not the full tile.
                    x8r = xt8.rearrange("(k p) t -> p k t", p=128)
                    xt8_sb = xtp.tile([128, NCH, TT], F8, tag="xts8",
                                      name=f"xts8_{j}")
                    for kk in range(0, NCH, 2):
                        nc.sync.dma_start(
                            out=xt8_sb[:, kk:kk + 2, :],
                            in_=x8r[:, kk:kk + 2, j * TT:(j + 1) * TT])
                    state["xt8"] = xt8_sb

                def do_load16():
                    xr = xt.rearrange("(k p) t -> p k t", p=128)
                    xt_sb = xtp.tile([128, NCH, TT], BF16, tag="xts",
                                     name=f"xts{j}")
                    for kk in range(0, NCH, 4):
                        nc.sync.dma_start(
                            out=xt_sb[:, kk:kk + 4, :],
                            in_=xr[:, kk:kk + 4, j * TT:(j + 1) * TT])
                    state["xt"] = xt_sb

                def do_qk(name, g, part):
                    # fp8e4 DoubleRow: two contraction chunks per matmul.
                    # Split into two pending chunks so each PE-queue insert
                    # is small (a big insert blocks the attention chain
                    # head-of-line and lets HAM re-throttle during the gap).
                    if part == 0:
                        pp = mixps.tile([128, TT], F32, tag="mix",
                                        name=f"pp_{name}{g}_{j}")
                        state[("pp", name, g)] = pp
                    else:
                        pp = state.pop(("pp", name, g))
                    for kk in (part * 4, part * 4 + 2):
                        nc.tensor.matmul(
                            pp,
                            lhsT=w_sb[name][:, kk:kk + 2, g * 128:(g + 1) * 128],
                            rhs=state["xt8"][:, kk:kk + 2, :],
                            start=(kk == 0), stop=(kk == NCH - 2),
                            skip_group_check=True,
                            perf_mode=mybir.MatmulPerfMode.DoubleRow,
                        )
                    if part == 1:
                        dst = QT if name == "q" else KT
                        ecost["v"] += TT + OV_V
                        nc.vector.tensor_scalar(
                            dst[:, g, j * TT:(j + 1) * TT], pp,
                            1.0 / WS, None, mybir.AluOpType.mult)

                def do_v(tb2, half, part):
                    if (half, part) == (0, 0):
                        vp = mixps.tile([128, TT], F32, tag="mix",
                                        name=f"vp{tb2}_{j}")
                        state[("vp", tb2)] = vp
                    else:
                        vp = state[("vp", tb2)]
                    tl = tb2 * 2 + half
                    for kk in range(part * 4, part * 4 + 4):
                        nc.tensor.matmul(
                            vp[:, half * 256:(half + 1) * 256],
                            lhsT=state["xt"][:, kk, tl * 128:(tl + 1) * 128],
                            rhs=wv_sb[:, kk, :],
                            start=(kk == 0), stop=(kk == NCH - 1),
                            skip_group_check=True,
                        )
                    if (half, part) == (1, 1):
                        del state[("vp", tb2)]
                        gtb = j * 4 + tb2 * 2
                        ecost["v"] += TT + OV_V
                        nc.vector.tensor_copy(
                            V[:, gtb:gtb + 2, :],
                            vp.rearrange("p (a b) -> p a b", a=2))

                def qk_ops(name, g):
                    return [lambda: do_qk(name, g, 0),
                            lambda: do_qk(name, g, 1)]

                def v_ops(tb2):
                    return [lambda h=h, p=p: do_v(tb2, h, p)
                            for h in range(2) for p in range(2)]

                ops = (qk_ops("q", 0) + qk_ops("q", 1)
                       + qk_ops("k", 0) + qk_ops("k", 1)
                       + v_ops(0) + v_ops(1))
                return do_load8, do_load16, ops

            def emit_attention(j, pending):
                n_sb = 4 * (j + 1)
                av_ps = {g: avps.tile([128, TT], F32, tag="avps",
                                      name=f"av{j}_{g}")
                         for g in range(NPAIR)}
                sm_ps = smps.tile([128, TT], F32, tag="smps", name=f"smps{j}")
                eg = {}
                LAG = 2

                def emit_scores(sb):
                    off = max(0, (sb - 4 * j) * SB)
                    for g in range(NPAIR):
                        S = spsum.tile([128, 2, TT], F32, tag="spsum",
                                       name=f"s{j}_{sb}_{g}")
                        for hh in range(2):
                            hp = slice(hh * 64, (hh + 1) * 64)
                            nc.tensor.matmul(
                                S[:, hh, off:],
                                lhsT=KT[hp, g, sb * SB:(sb + 1) * SB],
                                rhs=QT[hp, g, j * TT + off:(j + 1) * TT],
                                start=True, stop=True,
                            )
                        e = epool.tile([128, 2, TT], BF16, tag="e",
                                       name=f"e{j}_{sb}_{g}")
                        # short softmax rows (t<128) live in tile 0's s-block
                        # 0: keep that block exact on ScalarE; everything else
                        # is Schraudolph-eligible and greedily balanced.
                        if j == 0 and sb == 0:
                            eng = "s"
                            ecost["s"] += 2 * (TT - off) + OV_S
                        else:
                            eng = pick_engine(2 * (TT - off), slot=g)
                        if eng == "s":
                            nc.scalar.activation(
                                out=e[:, :, off:], in_=S[:, :, off:],
                                func=mybir.ActivationFunctionType.Exp)
                        else:
                            nc.vector.tensor_scalar(
                                e.bitcast(I16)[:, :, off:],
                                S[:, :, off:],
                                SCH_K, SCH_B,
                                mybir.AluOpType.mult, mybir.AluOpType.add)
                        if sb >= 4 * j:  # diagonal block: causal triangle
                            # on GpSimd: otherwise-idle engine, keeps DVE free
                            nc.gpsimd.affine_select(
                                out=e[:, :, off:off + SB],
                                in_=e[:, :, off:off + SB],
                                compare_op=mybir.AluOpType.is_ge,
                                fill=0.0, base=0,
                                pattern=[[0, 2], [1, SB]],
                                channel_multiplier=-1,
                            )
                        eg[(sb, g)] = e

                def emit_av(sb):
                    off = max(0, (sb - 4 * j) * SB)
                    es = {g: eg.pop((sb, g)) for g in range(NPAIR)}
                    for g in range(NPAIR):
                        for hh in range(2):
                            h = 2 * g + hh
                            nc.tensor.matmul(
                                av_ps[g][hh * 64:(hh + 1) * 64, off:],
                                lhsT=V[:, sb, h * 64:(h + 1) * 64],
                                rhs=es[g][:, hh, off:],
                                start=(sb == 0), stop=(sb == n_sb - 1),
                                skip_group_check=True,
                            )
                    for g in range(NPAIR):
                        for hh in range(2):
                            h = 2 * g + hh
                            nc.tensor.matmul(
                                sm_ps[32 * h:32 * h + 1, off:],
                                lhsT=ones1,
                                rhs=es[g][:, hh, off:],
                                start=(sb == 0), stop=(sb == n_sb - 1),
                                skip_group_check=True,
                                tile_position=(0, 32 * h),
                            )

                n_periods = n_sb + LAG
                n_pend = len(pending)
                popped = 0
                # Drain all pending work BEFORE the tail periods: anything
                # queued between this tile's last scores and its final AV
                # matmuls delays the av stop -> delays the output drains ->
                # (via the ScalarE queue) stalls the next tile's exps and
                # lets the HAM clock-gate re-throttle the PE.
                ramp = max(n_sb - LAG - 1, 4)
                for sb in range(n_periods):
                    if sb < n_sb:
                        emit_scores(sb)
                    want = min(n_pend, (n_pend * (sb + 1)) // ramp)
                    # the first pending items are the next xt load and the
                    # PREVIOUS tile's output drains; force them out in the
                    # first two periods -- a not-yet-drained av/sums PSUM
                    # bank would stall this tile's first AV matmul at the
                    # head of the PE queue, blocking everything behind it.
                    if sb == 0:
                        want = max(want, min(n_pend, 5))
                    elif sb == 1:
                        want = max(want, min(n_pend, 7))
                    while popped < want:
                        pending[popped]()
                        popped += 1
                    if sb >= LAG:
                        emit_av(sb - LAG)
                assert popped == n_pend

                # Output drains, returned as closures and emitted early in
                # the NEXT tile's attention (after its first exp) so the
                # ScalarE copies don't sit ahead of that tile's first exps
                # in the queue. They run on ScalarE (idle vs DVE; closer to
                # PSUM).
                last = j == nj - 1

                def drain_y(g):
                    y_sb = ysbp.tile([128, TT], F32, tag="ysb",
                                     name=f"y{j}_{g}")
                    # final tile: split across engines to shorten the tail
                    if last and g == 1:
                        ecost["v"] += TT + OV_V
                        nc.vector.tensor_copy(y_sb, av_ps[g])
                    else:
                        ecost["s"] += TT + OV_S
                        nc.scalar.copy(y_sb, av_ps[g])
                    nc.sync.dma_start(
                        out=yt[g * 128:(g + 1) * 128, j * TT:(j + 1) * TT],
                        in_=y_sb)

                def drain_sm():
                    # sums live on strided partitions {0,32,64,96}; engines
                    # can't compact partitions, so do ONE full-partition
                    # copy (the unused lanes carry garbage, harmless) and
                    # let 4 single-partition DMAs pick out the used rows.
                    sm_sb = ysbp.tile([128, TT], F32, tag="smsb",
                                      name=f"sm{j}")
                    ecost["s"] += TT + OV_S
                    nc.scalar.copy(sm_sb, sm_ps)
                    for h in range(HPC):
                        nc.sync.dma_start(
                            out=sm[j:j + 1, h:h + 1, :],
                            in_=sm_sb[32 * h:32 * h + 1, :])

                return [lambda: drain_y(0), lambda: drain_y(1), drain_sm]

            ld0_8, ld0_16, ops0 = proj_closures(0)
            ld0_8()
            nc.sync.dma_start(
                out=wq_sb, in_=wq.rearrange("(k p) d -> p k d", p=128))
            sm_ps_warm = smps.tile([128, TT], F32, tag="smps", name="smwarm")
            # PE warm-up: ~5us of dependency-free tiny matmuls during the
            # DMA fill so the HAM clock-gate is at 8/8 when the first
            # projection matmul issues (cold K=4/8 costs 2x for ~10us).
            # Writes scratch into the sums bank; attention later overwrites
            # it via start=True.
            for _ in range(240):
                nc.tensor.matmul(sm_ps_warm[0:1, 0:1], lhsT=ones1,
                                 rhs=ones1, start=True, stop=True,
                                 skip_group_check=True)
            # DMA ring order: fp8 xt (done), wq, wk, THEN the bf16 xt and
            # wv -- q/k proj (which gates the fill) only waits on the first
            # 2.5MB; the v path streams in behind it.
            nc.sync.dma_start(
                out=wk_sb, in_=wk.rearrange("(k p) d -> p k d", p=128))
            for op in ops0[0:4]:    # q projections (both pairs)
                op()
            ld0_16()
            for op in ops0[4:8]:    # k projections
                op()
            nc.sync.dma_start(
                out=wv_sb, in_=wv.rearrange("(k p) d -> p k d", p=128))
            for op in ops0[8:16]:   # v projections
                op()
            drains = []
            for j in range(nj):
                if j + 1 < nj:
                    ld8n, ld16n, opsn = proj_closures(j + 1)
                    nxt = [ld8n, ld16n] + drains + opsn
                else:
                    nxt = list(drains)
                drains = emit_attention(j, nxt)
            for dr in drains:
                dr()

    nc.compile()
    return nc


_CACHE = {}


def _get_runner():
    if "run" in _CACHE:
        return _CACHE["run"]

    import jax
    from jax.experimental.shard_map import shard_map
    from jax.sharding import Mesh, PartitionSpec
    from concourse import bass2jax
    from concourse.bass2jax import _bass_exec_p, install_neuronx_cc_hook

    nc = build_nc()
    install_neuronx_cc_hook()

    partition_name = (nc.partition_id_tensor.name
                      if nc.partition_id_tensor else None)
    in_names, out_names, out_avals, zero_outs = [], [], [], []
    for alloc in nc.m.functions[0].allocations:
        if not isinstance(alloc, mybir.MemoryLocationSet):
            continue
        name = alloc.memorylocations[0].name
        if alloc.kind == "ExternalInput":
            if name != partition_name:
                in_names.append(name)
        elif alloc.kind == "ExternalOutput":
            out_names.append(name)
            shape = tuple(alloc.tensor_shape)
            dtype = mybir.dt.np(alloc.dtype)
            out_avals.append(jax.core.ShapedArray(shape, dtype))
            zero_outs.append(np.zeros(shape, dtype))
    n_params = len(in_names)
    n_outs = len(out_avals)
    all_names = in_names + out_names
    if partition_name is not None:
        all_names = all_names + [partition_name]
    donate = tuple(range(n_params, n_params + n_outs))

    def _body(*args):
        operands = list(args)
        if partition_name is not None:
            operands.append(bass2jax.partition_id_tensor())
        outs = _bass_exec_p.bind(
            *operands,
            out_avals=tuple(out_avals),
            in_names=tuple(all_names),
            out_names=tuple(out_names),
            lowering_input_output_aliases=(),
            sim_require_finite=True,
            sim_require_nnan=True,
            nc=nc,
        )
        return tuple(outs)

    devices = jax.devices()[:NCORES]
    mesh = Mesh(np.asarray(devices), ("core",))
    in_specs = (PartitionSpec("core"),) * (n_params + n_outs)
    out_specs = (PartitionSpec("core"),) * n_outs
    sharded = jax.jit(
        shard_map(_body, mesh=mesh, in_specs=in_specs, out_specs=out_specs,
                  check_rep=False),
        donate_argnums=donate, keep_unused=True,
    )

    runner = {
        "nc": nc,
        "all_names": all_names,
        "sharded": sharded,
        "in_names": in_names,
        "out_names": out_names,
        "out_avals": out_avals,
        "zero_outs": zero_outs,
    }
    _CACHE["run"] = runner
    return runner


def _shard_inputs(x, Wq, Wk, Wv):
    """Per-core input dicts. Host-side layout prep only."""
    bf = mybir.dt.np(BF16)
    f8 = mybir.dt.np(F8)
    scale = float(C) ** -0.5
    maps = []
    for c in range(NCORES):
        b, hg = divmod(c, 4)
        hs = list(range(HPC * hg, HPC * hg + HPC))
        xtb = np.ascontiguousarray(np.transpose(x[b]))  # [C, T]
        wq2 = np.ascontiguousarray(
            (np.concatenate([Wq[h] for h in hs], axis=1)
             * (scale * WS)).astype(f8))
        wk2 = np.ascontiguousarray(
            (np.concatenate([Wk[h] for h in hs], axis=1) * WS).astype(f8))
        wv2 = np.ascontiguousarray(
            np.concatenate([Wv[h] for h in hs], axis=1).astype(bf))
        maps.append({"xt": xtb.astype(bf), "xt8": xtb.astype(f8),
                     "wq": wq2, "wk": wk2, "wv": wv2})
    return maps


def run_sharded(in_maps):
    """Run the 8-core NEFF once; returns list of per-core output dicts."""
    r = _get_runner()
    concat_in = [
        np.concatenate([in_maps[c][name] for c in range(NCORES)], axis=0)
        for name in r["in_names"]
    ]
    concat_zeros = [
        np.zeros((NCORES * z.shape[0], *z.shape[1:]), z.dtype)
        for z in r["zero_outs"]
    ]
    out_arrs = r["sharded"](*concat_in, *concat_zeros)
    return [
        {
            name: np.asarray(out_arrs[i]).reshape(
                NCORES, *r["out_avals"][i].shape)[c]
            for i, name in enumerate(r["out_names"])
        }
        for c in range(NCORES)
    ]


def kernel(x, Wq, Wk, Wv):
    x = np.asarray(x, dtype=np.float32)
    Wq = np.asarray(Wq, dtype=np.float32)
    Wk = np.asarray(Wk, dtype=np.float32)
    Wv = np.asarray(Wv, dtype=np.float32)
    in_maps = _shard_inputs(x, Wq, Wk, Wv)
    results = run_sharded(in_maps)
    outs = []
    for b in range(B):
        parts = []
        for hg in range(4):
            r = results[b * 4 + hg]
            ytc = np.asarray(r["yt"], dtype=np.float32)   # [256, T]
            smc = np.asarray(r["sm"], dtype=np.float32)   # [nj, 4, TT]
            smc = smc.transpose(1, 0, 2).reshape(HPC, T)  # [4, T]
            yn = ytc.reshape(HPC, D, T) / smc[:, None, :]
            parts.append(yn.reshape(HPC * D, T).T)        # [T, 256]
        outs.append(np.concatenate(parts, axis=1))        # [T, 1024]
    return np.ascontiguousarray(np.stack(outs)).astype(np.float32)

